# revision 31
# baseline (speedup 1.0000x reference)
"""Trainium2 Bass kernel for the 2-qubit quantum-circuit batch evaluation.

Reference semantics (per batch row, x = [x0, x1], scalar theta):
    state = RY(theta) @ CNOT @ (RY(x0)|0> ⊗ RY(x1)|0>)
    out = (<Z>, +1)/2 for each qubit.

Algebraically this reduces (product/half-angle identities) to:
    out0 = 0.5 + 0.5*cos(theta)*cos(x0) - 0.5*sin(theta)*sin(x0)*sin(x1)
    out1 = 0.5 + 0.5*cos(x0)*cos(x1)

Pure streaming map; per-core traffic is 8 MiB in + 8 MiB out and the 16-engine
DMA fabric sustains ~360 GB/s, so the roofline is ~47 us/core.  To get every
compute engine under that floor, the sin/cos argument reduction is fused into
single custom DVE ops (registered at import time into concourse's custom-DVE
table; the uops sha is computed on the spot so the pin can never drift):

  MAGIC_SIN_ARG:  ys = p - round(p),        p = x/(2pi)   -> sin(x) = Sin(2pi*ys)
  MAGIC_COS_ARG:  zs = 1/4 - |p - round(p)|               -> cos(x) = Sin(2pi*zs)
  MUL_AFFINE:     out = a*b*s0 + s1         (fused product+affine for out1)

round() is the f32 magic-number trick (add/sub 1.5*2^23) executed inside the
DVE uop cascade.  Per 128x4096 tile the engine loads are then
  DVE:    sin-arg + cos-arg + out1            ~11.1 us
  ACT:    two Sins + the out0 affine          ~9.7 us
  GPSIMD: sin0*sin1 product + out0 assembly   ~7.1 us
all below the ~11.7 us/tile DMA floor.

Sharding: pure data parallel over 8 NeuronCores; theta-derived scalars are
computed on host and passed as a tiny replicated [128, 4] constant tensor.
"""

import numpy as np

import concourse.bass as bass
import concourse.mybir as mybir
from concourse.alu_op_type import AluOpType
from concourse.bacc import Bacc
from concourse.tile import TileContext
from concourse import bass_utils

N_CORES = 8
B = 8388608
BC = B // N_CORES            # rows per core
ELEMS = BC * 2               # flat f32 elements per core
P = 128                      # SBUF partitions
F = 4096                     # max free elems per partition per tile
# Non-uniform schedule: small edge tiles shorten pipeline fill (first
# compute starts after a 0.25 MiB DMA instead of 2 MiB) and drain (the
# last tile's serial compute chain is 8x shorter).  Sums to ELEMS/P.
SCHEDULE = [512, 2048, 4096, 4096, 3584, 1536, 512]
assert sum(SCHEDULE) * P == ELEMS
TWO_PI = float(2 * np.pi)
INV_2PI = float(1.0 / (2 * np.pi))
HALF_PI = float(np.pi / 2)
MAGIC = float(1.5 * 2**23)   # f32 round-to-nearest-int magic constant

_CACHE = {}


def _register_custom_ops():
    """Register the three fused DVE ops into concourse's custom-op registry.

    Idempotent.  The uops sha is computed with the same lower() used at
    compile time, so the DveOp sha pin is always consistent."""
    from concourse import dve_ops as D
    from concourse.dve_spec import Spec, Src0, Src1, C0, C1, C2, Zero, lower, maxx
    from concourse.dve_uop import DveOpSpec

    if "MAGIC_SIN_ARG" in D._SUB_OPCODE_FOR_NAME:
        return

    def f32(v):
        return np.float32(v)

    def ref_sin_arg(in0, in1, s0, s1, imm2):
        p = (in0.astype(np.float32) * f32(s0)).astype(np.float32)
        t = (p + f32(s1)).astype(np.float32)
        k = (t - f32(s1)).astype(np.float32)
        return (p - k).astype(np.float32)

    def ref_cos_arg(in0, in1, s0, s1, imm2):
        p = (in0.astype(np.float32) * f32(s0)).astype(np.float32)
        t = (p + f32(s1)).astype(np.float32)
        k = (t - f32(s1)).astype(np.float32)
        y = (p - k).astype(np.float32)
        a = np.maximum(y, (-y).astype(np.float32))
        return (f32(imm2) - a).astype(np.float32)

    def ref_mul_affine(in0, in1, s0, s1, imm2):
        return (
            (in0.astype(np.float32) * in1 ).astype(np.float32) * f32(s0) + f32(s1)
        ).astype(np.float32)

    pA = Src0 * C0
    yA = pA - ((pA + C1) - C1)
    body_sin = yA

    pB = Src0 * C0
    yB = pB - ((pB + C1) - C1)
    body_cos = C2 - maxx(yB, Zero - yB)

    body_mul = Src0 * Src1 * C0 + C1

    defs = [
        ("MAGIC_SIN_ARG", body_sin, ref_sin_arg),
        ("MAGIC_COS_ARG", body_cos, ref_cos_arg),
        ("MUL_AFFINE", body_mul, ref_mul_affine),
    ]
    ops = {}
    for name, body, ref in defs:
        spec = Spec(body=body, reference=ref)
        row = max(D._SUB_OPCODE_FOR_NAME.values()) + 1
        assert row < 0x20, "custom DVE row field overflow"
        shas = {}
        for ver in ("v3", "v4"):
            uops = lower(spec, ver=ver)
            shas[ver] = DveOpSpec(
                name=name, opcode=row, uops=uops, rd1_en=D.has_src1(spec)
            ).sha(ver)
        op = D.DveOp(name, spec, subdim=False, uops_sha=shas)
        D.OPS.append(op)
        D.CUSTOM_DVE_SPECS[name] = spec
        D._SUB_OPCODE_FOR_NAME[name] = row
        ops[name] = op
    _CACHE["ops"] = ops


def _build_nc():
    _register_custom_ops()
    from concourse import dve_ops as D

    OP_SIN = next(o for o in D.OPS if o.name == "MAGIC_SIN_ARG")
    OP_COS = next(o for o in D.OPS if o.name == "MAGIC_COS_ARG")
    OP_MUL = next(o for o in D.OPS if o.name == "MUL_AFFINE")

    # Bacc (not raw Bass): its compile() pass splits multi-wait sync_info into
    # EventSemaphore instructions — TRN2 allows at most 1 wait per instruction.
    nc = Bacc()
    x = nc.dram_tensor("x", [BC, 2], mybir.dt.float32, kind="ExternalInput")
    consts = nc.dram_tensor("consts", [P, 4], mybir.dt.float32, kind="ExternalInput")
    out = nc.dram_tensor("out", [BC, 2], mybir.dt.float32, kind="ExternalOutput")

    # [P, ELEMS/P] per-partition flat views; tile i covers columns
    # [off_i, off_i + F_i).
    x_f = x[:].flatten().rearrange("(p e) -> p e", p=P)
    o_f = out[:].flatten().rearrange("(p e) -> p e", p=P)

    f32 = mybir.dt.float32
    Sin = mybir.ActivationFunctionType.Sin
    Ident = mybir.ActivationFunctionType.Identity

    with TileContext(nc) as tc:
        with tc.tile_pool(name="cpool", bufs=1) as cpool, \
             tc.tile_pool(name="io", bufs=3) as io, \
             tc.tile_pool(name="work", bufs=2) as work:
            bf16 = mybir.dt.bfloat16
            Abs = mybir.ActivationFunctionType.Abs
            ct = cpool.tile([P, 4], f32)
            hc = ct[:, 0:1]      # 0.5*cos(theta)
            ns = ct[:, 1:2]      # -0.5*sin(theta)
            half = ct[:, 2:3]    # 0.5
            halfpi = ct[:, 3:4]  # pi/2

            def head(off, Fi):
                """DMA-in + range reduction + trig for one tile."""
                H = Fi // 2
                xt = io.tile([P, F], f32, tag="xt")
                nc.sync.dma_start(out=xt[:, :Fi], in_=x_f[:, off:off + Fi])

                # Fused range reduction on DVE: ACT Sin is only accurate for
                # |arg| <= pi but x spans ~±18.  ys = x/(2pi) - round(x/(2pi))
                # in [-1/2, 1/2]; the custom op does the whole magic-rounding
                # cascade in one instruction.  It also deinterleaves the
                # (pair, lane) stream into lane PLANES so every downstream op
                # reads packed data: ys[p, l*H + j] = reduced x-lane-l of pair
                # j.  The plane dim must be OUTER in both APs (inner dim
                # count=2 costs a dim-crossing per element pair, ~+65%);
                # fine-grained stride-2 reads are cheap, so the input AP
                # walks lane 0 first, then lane 1.
                ys = work.tile([P, F], f32, tag="ys")
                ysw = ys[:, :Fi].rearrange("p (two h) -> p two h", two=2)
                xr = xt[:, :Fi].rearrange("p (h two) -> p two h", two=2)
                nc.vector._custom_dve(
                    OP_SIN, out=ysw, in0=xr, s0=INV_2PI, s1=MAGIC,
                )
                # sin(x) = Sin(2pi*ys); cos(x) = Sin(pi/2 - 2pi*|ys|).
                # All contiguous, all fresh tiles (in-place ACT is ~20%
                # slower).  S/C in bf16: 2-byte packed operands unlock the
                # DVE 2x/4x perf modes downstream; tolerance 2e-2 >> bf16 eps.
                S = work.tile([P, F], bf16, tag="S")
                ya = work.tile([P, F], bf16, tag="ya")
                C = work.tile([P, F], bf16, tag="C")
                nc.scalar.activation(S[:, :Fi], ys[:, :Fi], Sin, scale=TWO_PI)
                nc.scalar.activation(ya[:, :Fi], ys[:, :Fi], Abs)
                nc.scalar.activation(C[:, :Fi], ya[:, :Fi], Sin, bias=halfpi, scale=-TWO_PI)
                return off, Fi, S, C

            def tail(off, Fi, S, C):
                """Products + output assembly + DMA-out for one tile."""
                H = Fi // 2
                s0_, s1_ = S[:, 0:H], S[:, H:Fi]
                c0_, c1_ = C[:, 0:H], C[:, H:Fi]
                o = io.tile([P, F], f32, tag="o")
                ov = o[:, :Fi].rearrange("p (k two) -> p k two", two=2)

                # Products as packed-bf16 DVE TENSOR_TENSOR (2x_1p mode):
                # m = sin(x0)*sin(x1); g = cos(x0)*cos(x1)
                m = work.tile([P, F // 2], bf16, tag="m")
                g = work.tile([P, F // 2], bf16, tag="g")
                nc.vector.tensor_tensor(m[:, :H], s0_, s1_, AluOpType.mult)
                nc.vector.tensor_tensor(g[:, :H], c0_, c1_, AluOpType.mult)
                # a = 0.5*cos(theta)*cos(x0) + 0.5 (TS, 4x_2p all-bf16)
                a = work.tile([P, F // 2], bf16, tag="a")
                nc.vector.tensor_scalar(
                    a[:, :H], c0_, hc, half, AluOpType.mult, AluOpType.add,
                )
                # out1 = 0.5*g + 0.5 (TS, 2x_2p)
                nc.vector.tensor_scalar(
                    ov[:, :, 1], g[:, :H], 0.5, 0.5, AluOpType.mult, AluOpType.add,
                )
                # out0 = -0.5*sin(theta)*m + a (scalar_tensor_tensor)
                nc.vector.scalar_tensor_tensor(
                    ov[:, :, 0], m[:, :H], ns, a[:, :H], AluOpType.mult, AluOpType.add,
                )
                nc.sync.dma_start(out=o_f[:, off:off + Fi], in_=o[:, :Fi])

            # Software-pipelined: tile i's DVE tail is emitted AFTER tile
            # i+1's head, so while ACT runs tile i+1's trig chain the DVE is
            # never blocked in-order behind ops whose inputs aren't ready.
            nc.sync.dma_start(out=ct[:], in_=consts[:])
            pend = None
            off = 0
            for Fi in SCHEDULE:
                cur = head(off, Fi)
                if pend is not None:
                    tail(*pend)
                pend = cur
                off += Fi
            tail(*pend)
    nc.compile()
    return nc


def _run(in_maps, trace=False, trace_cores=None):
    if "nc" not in _CACHE:
        _CACHE["nc"] = _build_nc()
    return bass_utils.run_bass_kernel_spmd(
        _CACHE["nc"],
        in_maps,
        core_ids=list(range(N_CORES)),
        trace=trace,
        trace_cores=trace_cores,
    )


def kernel(x, theta, _trace=False, _trace_cores=None):
    x = np.ascontiguousarray(np.asarray(x, dtype=np.float32))
    theta = np.asarray(theta, dtype=np.float32)
    assert x.shape == (B, 2), x.shape

    th = float(theta.reshape(-1)[0])
    consts = np.empty((P, 4), dtype=np.float32)
    consts[:, 0] = 0.5 * np.cos(th)
    consts[:, 1] = -0.5 * np.sin(th)
    consts[:, 2] = 0.5
    consts[:, 3] = HALF_PI

    shards = x.reshape(N_CORES, BC, 2)
    in_maps = [{"x": shards[c], "consts": consts} for c in range(N_CORES)]

    res = _run(in_maps, trace=_trace, trace_cores=_trace_cores)
    _CACHE["last_results"] = res
    out = np.concatenate([res.results[c]["out"] for c in range(N_CORES)], axis=0)
    return out


# revision 32
# speedup vs baseline: 1.0685x; 1.0685x over previous
"""Trainium2 Bass kernel for the 2-qubit quantum-circuit batch evaluation.

Reference semantics (per batch row, x = [x0, x1], scalar theta):
    state = RY(theta) @ CNOT @ (RY(x0)|0> ⊗ RY(x1)|0>)
    out = (<Z>, +1)/2 for each qubit.

Algebraically this reduces (product/half-angle identities) to:
    out0 = 0.5 + 0.5*cos(theta)*cos(x0) - 0.5*sin(theta)*sin(x0)*sin(x1)
    out1 = 0.5 + 0.5*cos(x0)*cos(x1)

Pure streaming map; per-core traffic is 8 MiB in + 8 MiB out and the 16-engine
DMA fabric sustains ~360 GB/s, so the roofline is ~47 us/core.  To get every
compute engine under that floor, the sin/cos argument reduction is fused into
single custom DVE ops (registered at import time into concourse's custom-DVE
table; the uops sha is computed on the spot so the pin can never drift):

  MAGIC_SIN_ARG:  ys = p - round(p),        p = x/(2pi)   -> sin(x) = Sin(2pi*ys)
  MAGIC_COS_ARG:  zs = 1/4 - |p - round(p)|               -> cos(x) = Sin(2pi*zs)
  MUL_AFFINE:     out = a*b*s0 + s1         (fused product+affine for out1)

round() is the f32 magic-number trick (add/sub 1.5*2^23) executed inside the
DVE uop cascade.  Per 128x4096 tile the engine loads are then
  DVE:    sin-arg + cos-arg + out1            ~11.1 us
  ACT:    two Sins + the out0 affine          ~9.7 us
  GPSIMD: sin0*sin1 product + out0 assembly   ~7.1 us
all below the ~11.7 us/tile DMA floor.

Sharding: pure data parallel over 8 NeuronCores; theta-derived scalars are
computed on host and passed as a tiny replicated [128, 4] constant tensor.
"""

import numpy as np

import concourse.bass as bass
import concourse.mybir as mybir
from concourse.alu_op_type import AluOpType
from concourse.bacc import Bacc
from concourse.tile import TileContext
from concourse import bass_utils

N_CORES = 8
B = 8388608
BC = B // N_CORES            # rows per core
ELEMS = BC * 2               # flat f32 elements per core
P = 128                      # SBUF partitions
F = 4096                     # max free elems per partition per tile
# Non-uniform schedule: small edge tiles shorten pipeline fill (first
# compute starts after a 0.25 MiB DMA instead of 2 MiB) and drain (the
# last tile's serial compute chain is 8x shorter).  Sums to ELEMS/P.
SCHEDULE = [1024, 2048, 4096, 4096, 3072, 1536, 512]
assert sum(SCHEDULE) * P == ELEMS
TWO_PI = float(2 * np.pi)
INV_2PI = float(1.0 / (2 * np.pi))
HALF_PI = float(np.pi / 2)
MAGIC = float(1.5 * 2**23)   # f32 round-to-nearest-int magic constant

_CACHE = {}


def _register_custom_ops():
    """Register the three fused DVE ops into concourse's custom-op registry.

    Idempotent.  The uops sha is computed with the same lower() used at
    compile time, so the DveOp sha pin is always consistent."""
    from concourse import dve_ops as D
    from concourse.dve_spec import Spec, Src0, Src1, C0, C1, C2, Zero, lower, maxx
    from concourse.dve_uop import DveOpSpec

    if "MAGIC_SIN_ARG" in D._SUB_OPCODE_FOR_NAME:
        return

    def f32(v):
        return np.float32(v)

    def ref_sin_arg(in0, in1, s0, s1, imm2):
        p = (in0.astype(np.float32) * f32(s0)).astype(np.float32)
        t = (p + f32(s1)).astype(np.float32)
        k = (t - f32(s1)).astype(np.float32)
        return (p - k).astype(np.float32)

    def ref_cos_arg(in0, in1, s0, s1, imm2):
        p = (in0.astype(np.float32) * f32(s0)).astype(np.float32)
        t = (p + f32(s1)).astype(np.float32)
        k = (t - f32(s1)).astype(np.float32)
        y = (p - k).astype(np.float32)
        a = np.maximum(y, (-y).astype(np.float32))
        return (f32(imm2) - a).astype(np.float32)

    def ref_mul_affine(in0, in1, s0, s1, imm2):
        return (
            (in0.astype(np.float32) * in1 ).astype(np.float32) * f32(s0) + f32(s1)
        ).astype(np.float32)

    pA = Src0 * C0
    yA = pA - ((pA + C1) - C1)
    body_sin = yA

    pB = Src0 * C0
    yB = pB - ((pB + C1) - C1)
    body_cos = C2 - maxx(yB, Zero - yB)

    body_mul = Src0 * Src1 * C0 + C1

    defs = [
        ("MAGIC_SIN_ARG", body_sin, ref_sin_arg),
        ("MAGIC_COS_ARG", body_cos, ref_cos_arg),
        ("MUL_AFFINE", body_mul, ref_mul_affine),
    ]
    ops = {}
    for name, body, ref in defs:
        spec = Spec(body=body, reference=ref)
        row = max(D._SUB_OPCODE_FOR_NAME.values()) + 1
        assert row < 0x20, "custom DVE row field overflow"
        shas = {}
        for ver in ("v3", "v4"):
            uops = lower(spec, ver=ver)
            shas[ver] = DveOpSpec(
                name=name, opcode=row, uops=uops, rd1_en=D.has_src1(spec)
            ).sha(ver)
        op = D.DveOp(name, spec, subdim=False, uops_sha=shas)
        D.OPS.append(op)
        D.CUSTOM_DVE_SPECS[name] = spec
        D._SUB_OPCODE_FOR_NAME[name] = row
        ops[name] = op
    _CACHE["ops"] = ops


def _build_nc():
    _register_custom_ops()
    from concourse import dve_ops as D

    OP_SIN = next(o for o in D.OPS if o.name == "MAGIC_SIN_ARG")
    OP_COS = next(o for o in D.OPS if o.name == "MAGIC_COS_ARG")
    OP_MUL = next(o for o in D.OPS if o.name == "MUL_AFFINE")

    # Bacc (not raw Bass): its compile() pass splits multi-wait sync_info into
    # EventSemaphore instructions — TRN2 allows at most 1 wait per instruction.
    nc = Bacc()
    x = nc.dram_tensor("x", [BC, 2], mybir.dt.float32, kind="ExternalInput")
    consts = nc.dram_tensor("consts", [P, 4], mybir.dt.float32, kind="ExternalInput")
    out = nc.dram_tensor("out", [BC, 2], mybir.dt.float32, kind="ExternalOutput")

    # [P, ELEMS/P] per-partition flat views; tile i covers columns
    # [off_i, off_i + F_i).
    x_f = x[:].flatten().rearrange("(p e) -> p e", p=P)
    o_f = out[:].flatten().rearrange("(p e) -> p e", p=P)

    f32 = mybir.dt.float32
    Sin = mybir.ActivationFunctionType.Sin
    Ident = mybir.ActivationFunctionType.Identity

    with TileContext(nc) as tc:
        with tc.tile_pool(name="cpool", bufs=1) as cpool, \
             tc.tile_pool(name="io", bufs=3) as io, \
             tc.tile_pool(name="work", bufs=2) as work:
            bf16 = mybir.dt.bfloat16
            Abs = mybir.ActivationFunctionType.Abs
            ct = cpool.tile([P, 4], f32)
            hc = ct[:, 0:1]      # 0.5*cos(theta)
            ns = ct[:, 1:2]      # -0.5*sin(theta)
            half = ct[:, 2:3]    # 0.5
            halfpi = ct[:, 3:4]  # pi/2

            def head(off, Fi):
                """DMA-in + range reduction + trig for one tile."""
                H = Fi // 2
                xt = io.tile([P, F], f32, tag="xt")
                nc.sync.dma_start(out=xt[:, :Fi], in_=x_f[:, off:off + Fi])

                # Fused range reduction on DVE: ACT Sin is only accurate for
                # |arg| <= pi but x spans ~±18.  ys = x/(2pi) - round(x/(2pi))
                # in [-1/2, 1/2]; the custom op does the whole magic-rounding
                # cascade in one instruction.  It also deinterleaves the
                # (pair, lane) stream into lane PLANES so every downstream op
                # reads packed data: ys[p, l*H + j] = reduced x-lane-l of pair
                # j.  The plane dim must be OUTER in both APs (inner dim
                # count=2 costs a dim-crossing per element pair, ~+65%);
                # fine-grained stride-2 reads are cheap, so the input AP
                # walks lane 0 first, then lane 1.
                ys = work.tile([P, F], f32, tag="ys")
                ysw = ys[:, :Fi].rearrange("p (two h) -> p two h", two=2)
                xr = xt[:, :Fi].rearrange("p (h two) -> p two h", two=2)
                nc.vector._custom_dve(
                    OP_SIN, out=ysw, in0=xr, s0=INV_2PI, s1=MAGIC,
                )
                # sin(x) = Sin(2pi*ys); cos(x) = Sin(pi/2 - 2pi*|ys|).
                # All contiguous, all fresh tiles (in-place ACT is ~20%
                # slower).  S/C in bf16: 2-byte packed operands unlock the
                # DVE 2x/4x perf modes downstream; tolerance 2e-2 >> bf16 eps.
                S = work.tile([P, F], bf16, tag="S")
                ya = work.tile([P, F], bf16, tag="ya")
                C = work.tile([P, F], bf16, tag="C")
                nc.scalar.activation(S[:, :Fi], ys[:, :Fi], Sin, scale=TWO_PI)
                nc.scalar.activation(ya[:, :Fi], ys[:, :Fi], Abs)
                nc.scalar.activation(C[:, :Fi], ya[:, :Fi], Sin, bias=halfpi, scale=-TWO_PI)
                return off, Fi, S, C

            def tail(off, Fi, S, C):
                """Products + output assembly + DMA-out for one tile."""
                H = Fi // 2
                s0_, s1_ = S[:, 0:H], S[:, H:Fi]
                c0_, c1_ = C[:, 0:H], C[:, H:Fi]
                o = io.tile([P, F], f32, tag="o")
                ov = o[:, :Fi].rearrange("p (k two) -> p k two", two=2)

                # Products as packed-bf16 DVE TENSOR_TENSOR (2x_1p mode):
                # m = sin(x0)*sin(x1); g = cos(x0)*cos(x1)
                m = work.tile([P, F // 2], bf16, tag="m")
                g = work.tile([P, F // 2], bf16, tag="g")
                nc.vector.tensor_tensor(m[:, :H], s0_, s1_, AluOpType.mult)
                nc.vector.tensor_tensor(g[:, :H], c0_, c1_, AluOpType.mult)
                # a = 0.5*cos(theta)*cos(x0) + 0.5 (TS, 4x_2p all-bf16)
                a = work.tile([P, F // 2], bf16, tag="a")
                nc.vector.tensor_scalar(
                    a[:, :H], c0_, hc, half, AluOpType.mult, AluOpType.add,
                )
                # out1 = 0.5*g + 0.5 (TS, 2x_2p)
                nc.vector.tensor_scalar(
                    ov[:, :, 1], g[:, :H], 0.5, 0.5, AluOpType.mult, AluOpType.add,
                )
                # out0 = -0.5*sin(theta)*m + a (scalar_tensor_tensor)
                nc.vector.scalar_tensor_tensor(
                    ov[:, :, 0], m[:, :H], ns, a[:, :H], AluOpType.mult, AluOpType.add,
                )
                nc.sync.dma_start(out=o_f[:, off:off + Fi], in_=o[:, :Fi])

            # Software-pipelined: tile i's DVE tail is emitted AFTER tile
            # i+1's head, so while ACT runs tile i+1's trig chain the DVE is
            # never blocked in-order behind ops whose inputs aren't ready.
            nc.sync.dma_start(out=ct[:], in_=consts[:])
            pend = None
            off = 0
            for Fi in SCHEDULE:
                cur = head(off, Fi)
                if pend is not None:
                    tail(*pend)
                pend = cur
                off += Fi
            tail(*pend)
    nc.compile()
    return nc


def _run(in_maps, trace=False, trace_cores=None):
    if "nc" not in _CACHE:
        _CACHE["nc"] = _build_nc()
    return bass_utils.run_bass_kernel_spmd(
        _CACHE["nc"],
        in_maps,
        core_ids=list(range(N_CORES)),
        trace=trace,
        trace_cores=trace_cores,
    )


def kernel(x, theta, _trace=False, _trace_cores=None):
    x = np.ascontiguousarray(np.asarray(x, dtype=np.float32))
    theta = np.asarray(theta, dtype=np.float32)
    assert x.shape == (B, 2), x.shape

    th = float(theta.reshape(-1)[0])
    consts = np.empty((P, 4), dtype=np.float32)
    consts[:, 0] = 0.5 * np.cos(th)
    consts[:, 1] = -0.5 * np.sin(th)
    consts[:, 2] = 0.5
    consts[:, 3] = HALF_PI

    shards = x.reshape(N_CORES, BC, 2)
    in_maps = [{"x": shards[c], "consts": consts} for c in range(N_CORES)]

    res = _run(in_maps, trace=_trace, trace_cores=_trace_cores)
    _CACHE["last_results"] = res
    out = np.concatenate([res.results[c]["out"] for c in range(N_CORES)], axis=0)
    return out
